# revision 39
# baseline (speedup 1.0000x reference)
"""Trainium2 Bass kernel for nn_MoCo_4810363372846 (retrieval_knn).

Computation (see harness reference):
    h    = relu(im_q @ W1 + b1)            [B, 2048]
    q    = (h @ W2 + b2) row-normalized    [B, 128]
    dist = mean_j sqrt((q_i-k_j) invD (q_i-k_j)^T)  over 64 sampled queue cols
    top-63 (excluding the max) rows of dist gate a masked write into
    output[:, 2:4].

Strategy:
  * Data-parallel over the B=16384 rows: 8 NeuronCores x 2048 rows each.
    Weights / invD / sampled-queue constants are replicated.
  * Host pre-quantizes: X -> e4m3 feature-major [128, 16, rows];
    W1*8192 -> e4m3 [n, 128, 16, 128]; W2*8192 -> e4m3.  Both GEMMs run
    as fp8 DoubleRow matmuls (4x the fp22 PE rate), hidden activations
    stored e4m3, and the Mahalanobis tail in fp22 (f32r).  The serial
    normalize/Mahalanobis chain of each chunk is woven between the next
    chunk's GEMM groups so the in-order PE never stalls on it.  Device
    output: dist row [1, 2048] per core.
  * The fp8 pipeline perturbs dist by <~2.3e-2 (measured).  On host:
    gather the 8 dist shards, exactly recompute all rows within WINDOW of
    the top-64 threshold (and of the max), with an adaptive widen-and-
    retry guard, stable-argsort, build the row mask, and apply the masked
    write to output columns 2/3.
"""

import functools
import os

import numpy as np

# diagnosis switches (dev only; default off)
NO_WEAVE = os.environ.get("KERNEL_NO_WEAVE") == "1"
BF16_HT = os.environ.get("KERNEL_BF16_HT") == "1"

B, DIM_MLP, DIM, KQ, NUM = 16384, 2048, 128, 16384, 64
NCORES = 8
BL = B // NCORES  # 2048 rows per core
MC = 1024         # batch-chunk processed per pipeline pass
NCH = BL // MC
NH = 512          # matmul moving-operand free dim (one PSUM bank of fp32)
MH = MC // NH
P = 128
K16 = DIM_MLP // P  # 16 contraction sub-tiles
SW = 8192.0         # host-side W1 quantization scale (|W1|*SW <= 181 < 240)

# dist window (absolute units) around the top-64 / top-1 thresholds whose
# rows get an exact host-side recompute; ~2x the observed max fp8 error.
WINDOW = 4.5e-2


@functools.lru_cache(maxsize=None)
def _build_nc(reps=1):
    import concourse.mybir as mybir
    import concourse.tile as tile
    from concourse import bacc

    f32 = mybir.dt.float32
    f32r = mybir.dt.float32r
    bf16 = mybir.dt.bfloat16
    f8 = mybir.dt.float8e4
    u8 = mybir.dt.uint8
    AF = mybir.ActivationFunctionType
    DR = mybir.MatmulPerfMode.DoubleRow

    nc = bacc.Bacc(None, target_bir_lowering=False)

    x8 = nc.declare_dram_parameter("x8", [P, K16, BL], u8, isOutput=False)
    w1 = nc.declare_dram_parameter("w1", [K16, P, K16, P], u8, isOutput=False)
    w2q = nc.declare_dram_parameter("w2q", [P, K16, P], u8, isOutput=False)
    w2h = (
        nc.declare_dram_parameter("w2h", [P, K16, P], bf16, isOutput=False)
        if BF16_HT
        else None
    )
    b1t = nc.declare_dram_parameter("b1t", [P, K16], f32, isOutput=False)
    b2t = nc.declare_dram_parameter("b2t", [P, 1], f32, isOutput=False)
    invd = nc.declare_dram_parameter("invd", [P, P], f32, isOutput=False)
    ct = nc.declare_dram_parameter("ct", [P, NUM], f32, isOutput=False)
    colc = nc.declare_dram_parameter("colc", [P, 3], f32, isOutput=False)
    rowc = nc.declare_dram_parameter("rowc", [1, NH + NUM + P], f32, isOutput=False)
    dist = nc.declare_dram_parameter("dist", [1, BL], f32, isOutput=True)

    with tile.TileContext(nc) as tc:
        with (
            tc.tile_pool(name="const", bufs=1) as constp,
            tc.tile_pool(name="w1p", bufs=1) as w1p,
            tc.tile_pool(name="xin", bufs=2) as xinp,
            tc.tile_pool(name="ht", bufs=2) as htp,
            tc.tile_pool(name="dsb", bufs=2) as dsbp,
            tc.tile_pool(name="ps_h", bufs=4, space="PSUM") as ps_h,
            tc.tile_pool(name="ps_q", bufs=2, space="PSUM") as ps_q,
            tc.tile_pool(name="ps_d", bufs=2, space="PSUM") as ps_d,
        ):
            # allocate const tiles now; their DMAs are emitted AFTER the
            # first chunk's weight/activation DMAs so the PE's critical path
            # (w1n0 + x8[0]) is at the head of the DMA queue.
            colcs = constp.tile([P, 3], f32r)
            c2col = colcs[:NUM, 2:3].bitcast(f32)
            rowcs = constp.tile([1, NH + NUM + P], f32r)
            ones_k = colcs[:, 0:1]
            ones64s = colcs[:NUM, 1:2]
            negh64 = rowcs[:, NH : NH + NUM]
            ones_m32 = rowcs[:, NH + NUM :]
            b1s = constp.tile([P, K16], f32)
            b2s = constp.tile([P, 1], f32)
            invds = constp.tile([P, P], f32r)
            cts = constp.tile([P, NUM], f32r)
            if BF16_HT:
                w2s = constp.tile([P, K16, P], bf16)
            else:
                w2s = constp.tile([P, K16, P], f8)
            dist_sb = constp.tile([1, BL], f32)
            ht_dt = bf16 if BF16_HT else f8
            qt_scale = 1.0 if BF16_HT else 1.0 / SW

            def dma_consts():
                nc.sync.dma_start(b2s, b2t[:])
                nc.sync.dma_start(colcs, colc[:].bitcast(f32r))
                nc.sync.dma_start(rowcs, rowc[:].bitcast(f32r))
                nc.sync.dma_start(invds, invd[:].bitcast(f32r))
                nc.sync.dma_start(cts, ct[:].bitcast(f32r))
                if BF16_HT:
                    nc.sync.dma_start(w2s, w2h[:])
                else:
                    nc.sync.dma_start(w2s, w2q[:].bitcast(f8))

            # Deferred Mahalanobis-chain steps: each chunk's C/D phase is cut
            # into small steps that get woven between the NEXT chunk's B-phase
            # matmul groups, so the in-order PE never stalls on the serial
            # ACT/DVE chain.
            pending = []

            def emit_one():
                if pending:
                    pending.pop(0)()

            def cd_steps(c, htc):
                steps = []
                for m in range(MH):
                    st = {}

                    def s1(m=m, st=st, htc=htc):
                        pq = ps_q.tile([P, NH], f32, tag="pq")
                        if BF16_HT:
                            for k in range(K16):
                                nc.tensor.matmul(
                                    pq,
                                    w2s[:, k, :],
                                    htc[:, k, m * NH : (m + 1) * NH],
                                    start=(k == 0),
                                    stop=(k == K16 - 1),
                                )
                        else:
                            for kk in range(K16 // 2):
                                nc.tensor.matmul(
                                    pq,
                                    w2s[:, 2 * kk : 2 * kk + 2, :],
                                    htc[:, 2 * kk : 2 * kk + 2, m * NH : (m + 1) * NH],
                                    start=(kk == 0),
                                    stop=(kk == K16 // 2 - 1),
                                    perf_mode=DR,
                                )
                        qt = dsbp.tile([P, NH], f32, tag="qt")
                        nc.scalar.activation(
                            qt, pq, AF.Identity, bias=b2s[:, 0:1], scale=qt_scale
                        )
                        st["qt"] = qt

                    def s2(st=st):
                        qt = st["qt"]
                        sq = dsbp.tile([P, NH], f32r, tag="sq")
                        nc.vector.tensor_mul(sq, qt, qt)
                        pn = ps_d.tile([P, NH], f32, tag="pd")
                        nc.tensor.matmul(pn[:1, :], ones_k, sq)
                        st["pn"] = pn

                    def s3(st=st):
                        nrm = dsbp.tile([1, NH], f32, tag="nrm")
                        nc.scalar.activation(nrm, st["pn"][:1, :], AF.Sqrt)
                        s = dsbp.tile([1, NH], f32r, tag="s")
                        with nc.allow_low_precision("f32r==fp32 on DVE"):
                            nc.vector.reciprocal(s, nrm)
                        pb = ps_d.tile([P, NH], f32, tag="pd")
                        nc.tensor.matmul(pb, ones_m32, s)
                        st["pb"] = pb

                    def s4(st=st):
                        qn = dsbp.tile([P, NH], f32r, tag="qn")
                        nc.vector.tensor_mul(qn, st["qt"], st["pb"])
                        pu = ps_d.tile([P, NH], f32, tag="pd")
                        nc.tensor.matmul(pu, invds, qn)
                        st["qn"] = qn
                        st["pu"] = pu

                    def s5(st=st):
                        prod = dsbp.tile([P, NH], f32r, tag="prod")
                        nc.vector.tensor_mul(prod, st["qn"], st["pu"])
                        pr = ps_d.tile([P, NH], f32, tag="pd")
                        nc.tensor.matmul(pr[:1, :], ones_k, prod)
                        rsb = dsbp.tile([1, NH], f32r, tag="rsb")
                        nc.scalar.activation(rsb, pr[:1, :], AF.Identity)
                        st["rsb"] = rsb

                    def s6(st=st):
                        # psum = t - r/2 ; quad = -2*psum + c2 (c2 folded into
                        # the Sqrt activation's per-partition bias)
                        ptq = ps_d.tile([P, NH], f32, tag="pd")
                        nc.tensor.matmul(
                            ptq[:NUM, :], cts, st["qn"], start=True, stop=False
                        )
                        nc.tensor.matmul(
                            ptq[:NUM, :], negh64, st["rsb"], start=False, stop=True
                        )
                        sqq = dsbp.tile([NUM, NH], f32r, tag="sqq")
                        nc.scalar.activation(
                            sqq, ptq[:NUM, :], AF.Sqrt, scale=-2.0, bias=c2col
                        )
                        st["sqq"] = sqq

                    def s7(c=c, m=m, st=st):
                        pdd = ps_d.tile([P, NH], f32, tag="pd")
                        nc.tensor.matmul(pdd[:1, :], ones64s, st["sqq"])
                        o0 = c * MC + m * NH
                        nc.scalar.activation(
                            dist_sb[:, o0 : o0 + NH], pdd[:1, :], AF.Identity
                        )

                    steps += [s1, s2, s3, s4, s5, s6, s7]
                return steps

            def dma_x8(g):
                c = g % NCH
                pair = []
                for m in range(MH):
                    t = xinp.tile([P, K16, NH], f8, tag=f"x8{m}")
                    o0 = c * MC + m * NH
                    nc.sync.dma_start(t, x8.bitcast(f8)[:, :, o0 : o0 + NH])
                    pair.append(t)
                return pair

            G = reps * NCH

            def dma_w1(n):
                t = w1p.tile([P, K16, P], f8, tag=f"w1n{n}", name=f"w1n{n}")
                nc.sync.dma_start(t, w1[n].bitcast(f8))
                return t

            # head-critical DMA order: w1n0, first x8 halves, rest of w1,
            # then the small constants (first consumed only ~1 B-group in).
            nc.sync.dma_start(b1s, b1t[:])
            w1t = [dma_w1(0)]
            nxt = dma_x8(0)
            w1t += [dma_w1(n) for n in range(1, K16)]
            dma_consts()
            for g in range(G):
                c = g % NCH
                x8c = nxt
                if True:
                    if c == 0 and g > 0:
                        w1t = [dma_w1(n) for n in range(K16)]
                    if g + 1 < G:
                        nxt = dma_x8(g + 1)  # prefetch next chunk
                    # ---- h = relu((X8 @ W8)/SW + b1), stored e4m3 ----
                    htc = htp.tile([P, K16, MC], ht_dt, tag="htc")
                    for n in range(K16):
                        for m in range(MH):
                            ph = ps_h.tile([P, NH], f32, tag="ph")
                            for kk in range(K16 // 2):
                                nc.tensor.matmul(
                                    ph,
                                    w1t[n][:, 2 * kk : 2 * kk + 2, :],
                                    x8c[m][:, 2 * kk : 2 * kk + 2, :],
                                    start=(kk == 0),
                                    stop=(kk == K16 // 2 - 1),
                                    perf_mode=DR,
                                )
                            nc.scalar.activation(
                                htc[:, n, m * NH : (m + 1) * NH],
                                ph,
                                AF.Relu,
                                bias=b1s[:, n : n + 1],
                                scale=1.0 / SW,
                            )
                            emit_one()
                    pending.extend(cd_steps(c, htc))
                    if NO_WEAVE:
                        while pending:
                            emit_one()

            while pending:
                emit_one()
            nc.sync.dma_start(dist[:], dist_sb)

    nc.compile()
    return nc


def _host_constants(W1, b1, W2, b2, queue, invD, sample_idx):
    import ml_dtypes

    E4 = ml_dtypes.float8_e4m3
    qs = queue[:, sample_idx].T.astype(np.float64)  # [64, 128]
    iD = invD.astype(np.float64)
    ct = (iD @ qs.T).astype(np.float32)  # [128, 64]
    c2 = np.sum((qs @ iD) * qs, axis=1).astype(np.float32)[None, :]  # [1, 64]
    b1t = np.ascontiguousarray(
        b1.astype(np.float32).reshape(K16, P).T
    )  # [128, 16]; b1t[p, no] = b1[no*128+p]
    b2t = np.ascontiguousarray(b2.astype(np.float32).reshape(P, 1))
    colc = np.zeros((P, 3), np.float32)
    colc[:, 0] = 1.0
    colc[:, 1] = 1.0 / NUM
    colc[:NUM, 2] = c2[0]
    rowc = np.zeros((1, NH + NUM + P), np.float32)
    rowc[0, :NH] = -0.5
    rowc[0, NH : NH + NUM] = -0.5
    rowc[0, NH + NUM :] = 1.0
    # w1q[n, p, ko, m] = e4m3(W1[ko*128+p, n*128+m] * SW)
    w1q = np.ascontiguousarray(
        (W1 * np.float32(SW))
        .astype(E4)
        .reshape(K16, P, K16, P)
        .transpose(2, 1, 0, 3)
    ).view(np.uint8)
    # w2q[p, ko, d] = e4m3(W2[ko*128+p, d] * SW)
    w2q = np.ascontiguousarray(
        (W2 * np.float32(SW))
        .astype(E4)
        .reshape(K16, P, DIM)
        .transpose(1, 0, 2)
    ).view(np.uint8)
    # w2h[p, ko, d] = bf16(W2[ko*128+p, d])  (diagnosis variant)
    w2h = np.ascontiguousarray(
        W2.astype(ml_dtypes.bfloat16).reshape(K16, P, DIM).transpose(1, 0, 2)
    )
    return ct, c2, b1t, b2t, w1q, w2q, w2h, colc, rowc


def _host_x8(im_q):
    import ml_dtypes

    # x8[c, p, ko, b] = e4m3(im_q[c*BL + b, ko*128 + p])
    return np.ascontiguousarray(
        im_q.astype(ml_dtypes.float8_e4m3)
        .reshape(NCORES, BL, K16, P)
        .transpose(0, 3, 2, 1)
    ).view(np.uint8)


def per_core_inputs(inp):
    """Build the per-core input maps (host prep). Returns list of dicts."""
    im_q = np.ascontiguousarray(np.asarray(inp["im_q"], dtype=np.float32))
    W1 = np.ascontiguousarray(np.asarray(inp["W1"], dtype=np.float32))
    b1 = np.asarray(inp["b1"], dtype=np.float32)
    W2 = np.ascontiguousarray(np.asarray(inp["W2"], dtype=np.float32))
    b2 = np.asarray(inp["b2"], dtype=np.float32)
    queue = np.asarray(inp["queue"], dtype=np.float32)
    invD = np.ascontiguousarray(np.asarray(inp["invD"], dtype=np.float32))
    sample_idx = np.asarray(inp["sample_idx"])

    ct, c2, b1t, b2t, w1q, w2q, w2h, colc, rowc = _host_constants(
        W1, b1, W2, b2, queue, invD, sample_idx
    )
    x8 = _host_x8(im_q)
    in_maps = []
    for i in range(NCORES):
        in_maps.append(
            {
                "x8": x8[i],
                "w1": w1q,
                "w2q": w2q,
                **({"w2h": w2h} if BF16_HT else {}),
                "b1t": b1t,
                "b2t": b2t,
                "invd": invD,
                "ct": ct,
                "colc": colc,
                "rowc": rowc,
            }
        )
    return in_maps


def _exact_dist_rows(rows, im_q, W1, b1, W2, b2, qs64, iD64):
    X = im_q[rows].astype(np.float32)
    h = np.maximum(
        (X @ W1.astype(np.float32)).astype(np.float64) + b1.astype(np.float64), 0
    )
    q = h @ W2.astype(np.float64) + b2.astype(np.float64)
    q = q / np.maximum(np.linalg.norm(q, axis=1, keepdims=True), 1e-12)
    u = q @ iD64
    r = np.sum(u * q, axis=1)
    t = q @ (iD64 @ qs64.T)
    c2 = np.sum((qs64 @ iD64) * qs64, axis=1)
    quad = np.maximum(r[:, None] + c2[None, :] - 2 * t, 0)
    return np.sqrt(quad).mean(axis=1)


LAST_RESULTS = None  # for test harness introspection


def kernel(im_q, output, sample_idx, W1, b1, W2, b2, queue, invD):
    global LAST_RESULTS
    from concourse.bass_utils import run_bass_kernel_spmd

    inp = {
        "im_q": im_q, "W1": W1, "b1": b1, "W2": W2, "b2": b2,
        "queue": queue, "invD": invD, "sample_idx": sample_idx,
    }
    im_q = np.ascontiguousarray(np.asarray(im_q, dtype=np.float32))
    output = np.asarray(output, dtype=np.float32)
    W1 = np.ascontiguousarray(np.asarray(W1, dtype=np.float32))
    b1 = np.asarray(b1, dtype=np.float32)
    W2 = np.ascontiguousarray(np.asarray(W2, dtype=np.float32))
    b2 = np.asarray(b2, dtype=np.float32)
    queue = np.asarray(queue, dtype=np.float32)
    invD = np.ascontiguousarray(np.asarray(invD, dtype=np.float32))
    sample_idx = np.asarray(sample_idx)

    in_maps = per_core_inputs(inp)
    nc = _build_nc()
    res = run_bass_kernel_spmd(nc, in_maps, core_ids=list(range(NCORES)))
    LAST_RESULTS = res
    dist = np.concatenate(
        [np.asarray(res.results[i]["dist"]).reshape(BL) for i in range(NCORES)]
    ).astype(np.float64)

    # exact host recompute of rows near the top-64 inclusion boundary (and
    # the max-exclusion boundary) so fp8 error cannot flip the selected set
    qs64 = queue[:, sample_idx].T.astype(np.float64)
    iD64 = invD.astype(np.float64)
    win = WINDOW
    done = np.zeros(B, dtype=bool)
    for _attempt in range(4):
        thr = np.partition(dist, B - NUM)[B - NUM]
        top1 = dist.max()
        rows = np.nonzero(
            ((np.abs(dist - thr) <= win) | (dist >= top1 - win)) & ~done
        )[0]
        if rows.size:
            dist[rows] = _exact_dist_rows(
                rows, im_q, W1, b1, W2, b2, qs64, iD64
            )
            done[rows] = True
        # converged when every row within win/2 of the (updated) boundaries
        # has been exactly recomputed
        thr = np.partition(dist, B - NUM)[B - NUM]
        top1 = dist.max()
        chk = np.nonzero(
            ((np.abs(dist - thr) <= win / 2) | (dist >= top1 - win / 2)) & ~done
        )[0]
        if chk.size == 0:
            break

    order = np.argsort(dist, kind="stable")
    sel = order[-NUM:-1]
    row_mask = np.zeros(B, dtype=bool)
    row_mask[sel] = True
    cond = row_mask & ((np.abs(output[:, 2]) < 1.0) | (np.abs(output[:, 3]) < 1.0))
    out = output.copy()
    out[:, 2] = np.where(cond, np.float32(-5.0), output[:, 2])
    out[:, 3] = np.where(cond, np.float32(5.0), out[:, 3])
    return out


# revision 45
# speedup vs baseline: 1.6018x; 1.6018x over previous
"""Trainium2 Bass kernel for nn_MoCo_4810363372846 (retrieval_knn).

Computation (see harness reference):
    h    = relu(im_q @ W1 + b1)            [B, 2048]
    q    = (h @ W2 + b2) row-normalized    [B, 128]
    dist = mean_j sqrt((q_i-k_j) invD (q_i-k_j)^T)  over 64 sampled queue cols
    top-63 (excluding the max) rows of dist gate a masked write into
    output[:, 2:4].

Strategy:
  * Data-parallel over the B=16384 rows: 8 NeuronCores x 2048 rows each.
    Weights / invD / sampled-queue constants are replicated.
  * Host pre-quantizes: X -> e4m3 feature-major [128, 16, rows];
    W1*8192 -> e4m3 [n, 128, 16, 128]; W2*8192 -> e4m3.  Both GEMMs run
    as fp8 DoubleRow matmuls (4x the fp22 PE rate), hidden activations
    stored e4m3, and the Mahalanobis tail in fp22 (f32r).  The serial
    normalize/Mahalanobis chain of each chunk is woven between the next
    chunk's GEMM groups so the in-order PE never stalls on it.  Device
    output: dist row [1, 2048] per core.
  * The fp8 pipeline perturbs dist by <~2.3e-2 (measured).  On host:
    gather the 8 dist shards, exactly recompute all rows within WINDOW of
    the top-64 threshold (and of the max), with an adaptive widen-and-
    retry guard, stable-argsort, build the row mask, and apply the masked
    write to output columns 2/3.
"""

import functools
import os

import numpy as np

# diagnosis switches (dev only; default off)
NO_WEAVE = os.environ.get("KERNEL_NO_WEAVE") == "1"
BF16_HT = os.environ.get("KERNEL_BF16_HT") == "1"

B, DIM_MLP, DIM, KQ, NUM = 16384, 2048, 128, 16384, 64
NCORES = 8
BL = B // NCORES  # 2048 rows per core
MC = 1024         # batch-chunk processed per pipeline pass
NCH = BL // MC
NH = 512          # matmul moving-operand free dim (one PSUM bank of fp32)
MH = MC // NH
P = 128
K16 = DIM_MLP // P  # 16 contraction sub-tiles
SW = 8192.0         # host-side W1 quantization scale (|W1|*SW <= 181 < 240)

# dist window (absolute units) around the top-64 / top-1 thresholds whose
# rows get an exact host-side recompute; ~2x the observed max fp8 error.
WINDOW = 4.5e-2


@functools.lru_cache(maxsize=None)
def _build_nc(reps=1, hw_loop=False):
    import concourse.mybir as mybir
    import concourse.tile as tile
    from concourse import bacc

    f32 = mybir.dt.float32
    f32r = mybir.dt.float32r
    bf16 = mybir.dt.bfloat16
    f8 = mybir.dt.float8e4
    u8 = mybir.dt.uint8
    AF = mybir.ActivationFunctionType
    DR = mybir.MatmulPerfMode.DoubleRow

    nc = bacc.Bacc(None, target_bir_lowering=False)

    x8 = nc.declare_dram_parameter("x8", [P, K16, BL], u8, isOutput=False)
    w1 = nc.declare_dram_parameter("w1", [K16, P, K16, P], u8, isOutput=False)
    w2q = nc.declare_dram_parameter("w2q", [P, K16, P], u8, isOutput=False)
    w2h = (
        nc.declare_dram_parameter("w2h", [P, K16, P], bf16, isOutput=False)
        if BF16_HT
        else None
    )
    b1t = nc.declare_dram_parameter("b1t", [P, K16], f32, isOutput=False)
    b2t = nc.declare_dram_parameter("b2t", [P, 1], f32, isOutput=False)
    invd = nc.declare_dram_parameter("invd", [P, P], f32, isOutput=False)
    ct = nc.declare_dram_parameter("ct", [P, NUM], f32, isOutput=False)
    colc = nc.declare_dram_parameter("colc", [P, 3], f32, isOutput=False)
    rowc = nc.declare_dram_parameter("rowc", [1, NH + NUM + P], f32, isOutput=False)
    dist = nc.declare_dram_parameter("dist", [1, BL], f32, isOutput=True)

    with tile.TileContext(nc) as tc:
        with (
            tc.tile_pool(name="const", bufs=1) as constp,
            tc.tile_pool(name="w1p", bufs=1) as w1p,
            tc.tile_pool(name="xin", bufs=2) as xinp,
            tc.tile_pool(name="ht", bufs=2) as htp,
            tc.tile_pool(name="dsb", bufs=2) as dsbp,
            tc.tile_pool(name="ps_h", bufs=4, space="PSUM") as ps_h,
            tc.tile_pool(name="ps_q", bufs=2, space="PSUM") as ps_q,
            tc.tile_pool(name="ps_d", bufs=2, space="PSUM") as ps_d,
        ):
            # allocate const tiles now; their DMAs are emitted AFTER the
            # first chunk's weight/activation DMAs so the PE's critical path
            # (w1n0 + x8[0]) is at the head of the DMA queue.
            colcs = constp.tile([P, 3], f32r)
            c2col = colcs[:NUM, 2:3].bitcast(f32)
            rowcs = constp.tile([1, NH + NUM + P], f32r)
            ones_k = colcs[:, 0:1]
            ones64s = colcs[:NUM, 1:2]
            negh64 = rowcs[:, NH : NH + NUM]
            ones_m32 = rowcs[:, NH + NUM :]
            b1s = constp.tile([P, K16], f32)
            b2s = constp.tile([P, 1], f32)
            invds = constp.tile([P, P], f32r)
            cts = constp.tile([P, NUM], f32r)
            if BF16_HT:
                w2s = constp.tile([P, K16, P], bf16)
            else:
                w2s = constp.tile([P, K16, P], f8)
            dist_sb = constp.tile([1, BL], f32)
            ht_dt = bf16 if BF16_HT else f8
            qt_scale = 1.0 if BF16_HT else 1.0 / SW

            def dma_consts():
                nc.sync.dma_start(b2s, b2t[:])
                nc.sync.dma_start(colcs, colc[:].bitcast(f32r))
                nc.sync.dma_start(rowcs, rowc[:].bitcast(f32r))
                nc.sync.dma_start(invds, invd[:].bitcast(f32r))
                nc.sync.dma_start(cts, ct[:].bitcast(f32r))
                if BF16_HT:
                    nc.sync.dma_start(w2s, w2h[:])
                else:
                    nc.sync.dma_start(w2s, w2q[:].bitcast(f8))

            # Deferred Mahalanobis-chain steps: each chunk's C/D phase is cut
            # into small steps that get woven between the NEXT chunk's B-phase
            # matmul groups, so the in-order PE never stalls on the serial
            # ACT/DVE chain.
            pending = []

            def emit_one():
                if pending:
                    pending.pop(0)()

            def cd_steps(c, htc):
                steps = []
                for m in range(MH):
                    st = {}

                    def s1(m=m, st=st, htc=htc):
                        pq = ps_q.tile([P, NH], f32, tag="pq")
                        if BF16_HT:
                            for k in range(K16):
                                nc.tensor.matmul(
                                    pq,
                                    w2s[:, k, :],
                                    htc[:, k, m * NH : (m + 1) * NH],
                                    start=(k == 0),
                                    stop=(k == K16 - 1),
                                )
                        else:
                            for kk in range(K16 // 2):
                                nc.tensor.matmul(
                                    pq,
                                    w2s[:, 2 * kk : 2 * kk + 2, :],
                                    htc[:, 2 * kk : 2 * kk + 2, m * NH : (m + 1) * NH],
                                    start=(kk == 0),
                                    stop=(kk == K16 // 2 - 1),
                                    perf_mode=DR,
                                )
                        qt = dsbp.tile([P, NH], f32, tag="qt")
                        nc.scalar.activation(
                            qt, pq, AF.Identity, bias=b2s[:, 0:1], scale=qt_scale
                        )
                        st["qt"] = qt

                    def s2(st=st):
                        qt = st["qt"]
                        sq = dsbp.tile([P, NH], f32r, tag="sq")
                        nc.vector.tensor_mul(sq, qt, qt)
                        pn = ps_d.tile([P, NH], f32, tag="pd")
                        nc.tensor.matmul(pn[:1, :], ones_k, sq)
                        st["pn"] = pn

                    def s3(st=st):
                        nrm = dsbp.tile([1, NH], f32, tag="nrm")
                        nc.scalar.activation(nrm, st["pn"][:1, :], AF.Sqrt)
                        s = dsbp.tile([1, NH], f32r, tag="s")
                        with nc.allow_low_precision("f32r==fp32 on DVE"):
                            nc.vector.reciprocal(s, nrm)
                        pb = ps_d.tile([P, NH], f32, tag="pd")
                        nc.tensor.matmul(pb, ones_m32, s)
                        st["pb"] = pb

                    def s4(st=st):
                        qn = dsbp.tile([P, NH], f32r, tag="qn")
                        nc.vector.tensor_mul(qn, st["qt"], st["pb"])
                        pu = ps_d.tile([P, NH], f32, tag="pd")
                        nc.tensor.matmul(pu, invds, qn)
                        st["qn"] = qn
                        st["pu"] = pu

                    def s5(st=st):
                        prod = dsbp.tile([P, NH], f32r, tag="prod")
                        nc.vector.tensor_mul(prod, st["qn"], st["pu"])
                        pr = ps_d.tile([P, NH], f32, tag="pd")
                        nc.tensor.matmul(pr[:1, :], ones_k, prod)
                        rsb = dsbp.tile([1, NH], f32r, tag="rsb")
                        nc.scalar.activation(rsb, pr[:1, :], AF.Identity)
                        st["rsb"] = rsb

                    def s6(st=st):
                        # psum = t - r/2 ; quad = -2*psum + c2 (c2 folded into
                        # the Sqrt activation's per-partition bias)
                        ptq = ps_d.tile([P, NH], f32, tag="pd")
                        nc.tensor.matmul(
                            ptq[:NUM, :], cts, st["qn"], start=True, stop=False
                        )
                        nc.tensor.matmul(
                            ptq[:NUM, :], negh64, st["rsb"], start=False, stop=True
                        )
                        sqq = dsbp.tile([NUM, NH], f32r, tag="sqq")
                        nc.scalar.activation(
                            sqq, ptq[:NUM, :], AF.Sqrt, scale=-2.0, bias=c2col
                        )
                        st["sqq"] = sqq

                    def s7(c=c, m=m, st=st):
                        pdd = ps_d.tile([P, NH], f32, tag="pd")
                        nc.tensor.matmul(pdd[:1, :], ones64s, st["sqq"])
                        o0 = c * MC + m * NH
                        nc.scalar.activation(
                            dist_sb[:, o0 : o0 + NH], pdd[:1, :], AF.Identity
                        )

                    steps += [s1, s2, s3, s4, s5, s6, s7]
                return steps

            def dma_x8(g):
                c = g % NCH
                pair = []
                for m in range(MH):
                    t = xinp.tile([P, K16, NH], f8, tag=f"x8{m}")
                    o0 = c * MC + m * NH
                    nc.sync.dma_start(t, x8.bitcast(f8)[:, :, o0 : o0 + NH])
                    pair.append(t)
                return pair

            G = reps * NCH

            def dma_w1(n):
                t = w1p.tile([P, K16, P], f8, tag=f"w1n{n}", name=f"w1n{n}")
                nc.sync.dma_start(t, w1[n].bitcast(f8))
                return t

            def emit_chunk(c, w1t, x8c):
                # ---- h = relu((X8 @ W8)/SW + b1), stored e4m3 ----
                htc = htp.tile([P, K16, MC], ht_dt, tag="htc")
                for n in range(K16):
                    for m in range(MH):
                        ph = ps_h.tile([P, NH], f32, tag="ph")
                        for kk in range(K16 // 2):
                            nc.tensor.matmul(
                                ph,
                                w1t[n][:, 2 * kk : 2 * kk + 2, :],
                                x8c[m][:, 2 * kk : 2 * kk + 2, :],
                                start=(kk == 0),
                                stop=(kk == K16 // 2 - 1),
                                perf_mode=DR,
                            )
                        nc.scalar.activation(
                            htc[:, n, m * NH : (m + 1) * NH],
                            ph,
                            AF.Relu,
                            bias=b1s[:, n : n + 1],
                            scale=1.0 / SW,
                        )
                        emit_one()
                pending.extend(cd_steps(c, htc))
                if NO_WEAVE:
                    while pending:
                        emit_one()

            # head-critical DMA order: w1n0, first x8 halves, rest of w1,
            # then the small constants (first consumed only ~1 B-group in).
            nc.sync.dma_start(b1s, b1t[:])
            if hw_loop:
                dma_consts()
                with tc.For_i(0, reps, 1):
                    w1t = [dma_w1(n) for n in range(K16)]
                    xs = [dma_x8(c) for c in range(NCH)]
                    for c in range(NCH):
                        emit_chunk(c, w1t, xs[c])
                    while pending:
                        emit_one()
            else:
                w1t = [dma_w1(0)]
                nxt = dma_x8(0)
                w1t += [dma_w1(n) for n in range(1, K16)]
                dma_consts()
                for g in range(G):
                    c = g % NCH
                    x8c = nxt
                    if c == 0 and g > 0:
                        w1t = [dma_w1(n) for n in range(K16)]
                    if g + 1 < G:
                        nxt = dma_x8(g + 1)  # prefetch next chunk
                    emit_chunk(c, w1t, x8c)
                while pending:
                    emit_one()
            nc.sync.dma_start(dist[:], dist_sb)

    nc.compile()
    return nc


def _host_constants(W1, b1, W2, b2, queue, invD, sample_idx):
    import ml_dtypes

    E4 = ml_dtypes.float8_e4m3
    qs = queue[:, sample_idx].T.astype(np.float64)  # [64, 128]
    iD = invD.astype(np.float64)
    ct = (iD @ qs.T).astype(np.float32)  # [128, 64]
    c2 = np.sum((qs @ iD) * qs, axis=1).astype(np.float32)[None, :]  # [1, 64]
    b1t = np.ascontiguousarray(
        b1.astype(np.float32).reshape(K16, P).T
    )  # [128, 16]; b1t[p, no] = b1[no*128+p]
    b2t = np.ascontiguousarray(b2.astype(np.float32).reshape(P, 1))
    colc = np.zeros((P, 3), np.float32)
    colc[:, 0] = 1.0
    colc[:, 1] = 1.0 / NUM
    colc[:NUM, 2] = c2[0]
    rowc = np.zeros((1, NH + NUM + P), np.float32)
    rowc[0, :NH] = -0.5
    rowc[0, NH : NH + NUM] = -0.5
    rowc[0, NH + NUM :] = 1.0
    # w1q[n, p, ko, m] = e4m3(W1[ko*128+p, n*128+m] * SW)
    w1q = np.ascontiguousarray(
        (W1 * np.float32(SW))
        .astype(E4)
        .reshape(K16, P, K16, P)
        .transpose(2, 1, 0, 3)
    ).view(np.uint8)
    # w2q[p, ko, d] = e4m3(W2[ko*128+p, d] * SW)
    w2q = np.ascontiguousarray(
        (W2 * np.float32(SW))
        .astype(E4)
        .reshape(K16, P, DIM)
        .transpose(1, 0, 2)
    ).view(np.uint8)
    # w2h[p, ko, d] = bf16(W2[ko*128+p, d])  (diagnosis variant)
    w2h = np.ascontiguousarray(
        W2.astype(ml_dtypes.bfloat16).reshape(K16, P, DIM).transpose(1, 0, 2)
    )
    return ct, c2, b1t, b2t, w1q, w2q, w2h, colc, rowc


def _host_x8(im_q):
    import ml_dtypes

    # x8[c, p, ko, b] = e4m3(im_q[c*BL + b, ko*128 + p])
    return np.ascontiguousarray(
        im_q.astype(ml_dtypes.float8_e4m3)
        .reshape(NCORES, BL, K16, P)
        .transpose(0, 3, 2, 1)
    ).view(np.uint8)


def per_core_inputs(inp):
    """Build the per-core input maps (host prep). Returns list of dicts."""
    im_q = np.ascontiguousarray(np.asarray(inp["im_q"], dtype=np.float32))
    W1 = np.ascontiguousarray(np.asarray(inp["W1"], dtype=np.float32))
    b1 = np.asarray(inp["b1"], dtype=np.float32)
    W2 = np.ascontiguousarray(np.asarray(inp["W2"], dtype=np.float32))
    b2 = np.asarray(inp["b2"], dtype=np.float32)
    queue = np.asarray(inp["queue"], dtype=np.float32)
    invD = np.ascontiguousarray(np.asarray(inp["invD"], dtype=np.float32))
    sample_idx = np.asarray(inp["sample_idx"])

    ct, c2, b1t, b2t, w1q, w2q, w2h, colc, rowc = _host_constants(
        W1, b1, W2, b2, queue, invD, sample_idx
    )
    x8 = _host_x8(im_q)
    in_maps = []
    for i in range(NCORES):
        in_maps.append(
            {
                "x8": x8[i],
                "w1": w1q,
                "w2q": w2q,
                **({"w2h": w2h} if BF16_HT else {}),
                "b1t": b1t,
                "b2t": b2t,
                "invd": invD,
                "ct": ct,
                "colc": colc,
                "rowc": rowc,
            }
        )
    return in_maps


def _exact_dist_rows(rows, im_q, W1, b1, W2, b2, qs64, iD64):
    X = im_q[rows].astype(np.float32)
    h = np.maximum(
        (X @ W1.astype(np.float32)).astype(np.float64) + b1.astype(np.float64), 0
    )
    q = h @ W2.astype(np.float64) + b2.astype(np.float64)
    q = q / np.maximum(np.linalg.norm(q, axis=1, keepdims=True), 1e-12)
    u = q @ iD64
    r = np.sum(u * q, axis=1)
    t = q @ (iD64 @ qs64.T)
    c2 = np.sum((qs64 @ iD64) * qs64, axis=1)
    quad = np.maximum(r[:, None] + c2[None, :] - 2 * t, 0)
    return np.sqrt(quad).mean(axis=1)


LAST_RESULTS = None  # for test harness introspection
LAST_STATS = None  # recompute-row count + observed fp8 boundary error


def kernel(im_q, output, sample_idx, W1, b1, W2, b2, queue, invD):
    global LAST_RESULTS, LAST_STATS
    from concourse.bass_utils import run_bass_kernel_spmd

    inp = {
        "im_q": im_q, "W1": W1, "b1": b1, "W2": W2, "b2": b2,
        "queue": queue, "invD": invD, "sample_idx": sample_idx,
    }
    im_q = np.ascontiguousarray(np.asarray(im_q, dtype=np.float32))
    output = np.asarray(output, dtype=np.float32)
    W1 = np.ascontiguousarray(np.asarray(W1, dtype=np.float32))
    b1 = np.asarray(b1, dtype=np.float32)
    W2 = np.ascontiguousarray(np.asarray(W2, dtype=np.float32))
    b2 = np.asarray(b2, dtype=np.float32)
    queue = np.asarray(queue, dtype=np.float32)
    invD = np.ascontiguousarray(np.asarray(invD, dtype=np.float32))
    sample_idx = np.asarray(sample_idx)

    in_maps = per_core_inputs(inp)
    nc = _build_nc()
    res = run_bass_kernel_spmd(nc, in_maps, core_ids=list(range(NCORES)))
    LAST_RESULTS = res
    dist = np.concatenate(
        [np.asarray(res.results[i]["dist"]).reshape(BL) for i in range(NCORES)]
    ).astype(np.float64)

    # exact host recompute of rows near the top-64 inclusion boundary (and
    # the max-exclusion boundary) so fp8 error cannot flip the selected set
    qs64 = queue[:, sample_idx].T.astype(np.float64)
    iD64 = invD.astype(np.float64)
    win = WINDOW
    done = np.zeros(B, dtype=bool)
    max_err = 0.0
    for _attempt in range(4):
        thr = np.partition(dist, B - NUM)[B - NUM]
        top1 = dist.max()
        rows = np.nonzero(
            ((np.abs(dist - thr) <= win) | (dist >= top1 - win)) & ~done
        )[0]
        if rows.size:
            prev = dist[rows].copy()
            dist[rows] = _exact_dist_rows(
                rows, im_q, W1, b1, W2, b2, qs64, iD64
            )
            max_err = max(max_err, float(np.abs(dist[rows] - prev).max()))
            done[rows] = True
        # converged when every row within win/2 of the (updated) boundaries
        # has been exactly recomputed
        thr = np.partition(dist, B - NUM)[B - NUM]
        top1 = dist.max()
        chk = np.nonzero(
            ((np.abs(dist - thr) <= win / 2) | (dist >= top1 - win / 2)) & ~done
        )[0]
        if chk.size == 0:
            break

    LAST_STATS = {
        "recompute_rows": int(done.sum()),
        "max_fp8_err_at_boundary": max_err,
        "window": win,
    }
    order = np.argsort(dist, kind="stable")
    sel = order[-NUM:-1]
    row_mask = np.zeros(B, dtype=bool)
    row_mask[sel] = True
    cond = row_mask & ((np.abs(output[:, 2]) < 1.0) | (np.abs(output[:, 3]) < 1.0))
    out = output.copy()
    out[:, 2] = np.where(cond, np.float32(-5.0), output[:, 2])
    out[:, 3] = np.where(cond, np.float32(5.0), out[:, 3])
    return out
